# revision 4
# baseline (speedup 1.0000x reference)
"""CRF log-partition (forward algorithm) on 8 Trainium2 NeuronCores.

Strategy: data-parallel over batch (16 columns per core) PLUS a 2x serial-
depth cut by running the forward and backward recurrences simultaneously,
meeting in the middle.

    logZ_b = w^T (prod_{t=len-1..0} D_t Ef) a0,   D_t = diag(exp(u[t,b,:]))
           = g_m^T f_m
    f_{k+1} = D_k Ef^T... (forward),  g^(k) = Ef^T-apply (backward)

Device state X [128 partitions, 16 cols]: top 64 = Ef^T-prestate of the
forward pass, bottom 64 = backward state. One fused iteration:

    Y = X * V_k            (DVE elementwise, V holds softmax(u) rows)
    X' = blockdiag(Ef^T, Ef) @ Y     (PE matmul, one stationary forever)

K = 128 iterations instead of T = 256 -- the chain latency (~0.5us/step:
PE sbuf latency + sem + DVE PSUM access + ack + sem) is the bottleneck, so
halving the serial depth halves the span.  Per-(t,b) softmax normalization
is folded into V on the host (compensation C_b added back at the end),
which keeps all state magnitudes O(1) -- no on-device rescaling.  Each
column's Y is captured into Ycap at its two meeting-point iterations by an
off-critical-path predicated copy; the final dot product g_m^T f_m and the
log happen on the host.
"""

import numpy as np

T, B, N = 256, 128, 64
START_IDX, END_IDX = 1, 2
NCORES = 8
BC = B // NCORES   # 16 columns per core
K = T // 2         # fused iterations
CH = 16            # DMA chunks for V/Cp
CW = K * BC // CH  # chunk width in free elems (128)


def _build_nc():
    import concourse.bacc as bacc
    import concourse.mybir as mybir
    from concourse.tile import TileContext

    f32 = mybir.dt.float32
    u8 = mybir.dt.uint8

    nc = bacc.Bacc(None, target_bir_lowering=False)
    v_d = nc.dram_tensor("v", [2 * N, K * BC], f32, kind="ExternalInput")
    cp_d = nc.dram_tensor("cp", [2 * N, K * BC], u8, kind="ExternalInput")
    e2_d = nc.dram_tensor("e2", [2 * N, 2 * N], f32, kind="ExternalInput")
    x0_d = nc.dram_tensor("x0", [2 * N, BC], f32, kind="ExternalInput")
    o_d = nc.dram_tensor("out", [2 * N, BC], f32, kind="ExternalOutput")

    with TileContext(nc) as tc:
        with (
            tc.tile_pool(name="big", bufs=1) as big,
            tc.tile_pool(name="pp", bufs=2, space="PSUM") as pp,
        ):
            V = big.tile([2 * N, K * BC], f32, tag="V")
            Cp = big.tile([2 * N, K * BC], u8, tag="Cp")
            E2 = big.tile([2 * N, 2 * N], f32, tag="E2")
            X0 = big.tile([2 * N, BC], f32, tag="X0")
            Y0 = big.tile([2 * N, BC], f32, tag="Y0")
            Y1 = big.tile([2 * N, BC], f32, tag="Y1")
            Yc = big.tile([2 * N, BC], f32, tag="Yc")
            Ybufs = [Y0, Y1]

            nc.sync.dma_start(E2[:], e2_d[:])
            nc.sync.dma_start(X0[:], x0_d[:])
            nc.gpsimd.memset(Yc[:], 0.0)
            for ch in range(CH):
                sl = slice(ch * CW, (ch + 1) * CW)
                nc.sync.dma_start(V[:, sl], v_d[:, sl])
                nc.sync.dma_start(Cp[:, sl], cp_d[:, sl])

            Xprev = None
            for k in range(K):
                Yk = Ybufs[k % 2]
                ks = slice(k * BC, (k + 1) * BC)
                src = X0 if k == 0 else Xprev
                nc.vector.tensor_mul(Yk[:], src[:], V[:, ks])
                nc.vector.copy_predicated(Yc[:], Cp[:, ks], Yk[:])
                if k < K - 1:
                    Xp = pp.tile([2 * N, BC], f32, tag=f"X{k % 2}")
                    nc.tensor.matmul(Xp[:], E2[:], Yk[:], start=True, stop=True)
                    Xprev = Xp

            nc.sync.dma_start(o_d[:], Yc[:])
    nc.finalize()
    return nc


def _host_prep(unary, trans, lengths):
    """Vectorized host-side prep of V, Cp, X0, E2 and compensation C."""
    u = np.asarray(unary, np.float32)                 # [T, B, N]
    tr = np.asarray(trans, np.float32)[0]             # [to, fr]
    ln = np.asarray(lengths).astype(np.int64)         # [B]

    mx = u.max(axis=2)                                 # [T, B]
    e = np.exp(u - mx[:, :, None]).astype(np.float32)  # [T, B, N]
    s = e.sum(axis=2, dtype=np.float32)                # [T, B]
    P = (e / s[:, :, None]).astype(np.float32)         # softmax rows
    r = mx.astype(np.float64) + np.log(s.astype(np.float64))  # [T, B]

    m = (ln + 1) // 2                                  # [B] forward steps
    kk = np.arange(K)                                  # iteration index

    # top: V[0:64, k, b] = P[k, b, :] while k < m_b
    topmask = kk[None, :] < m[:, None]                 # [B, K]
    Vtop = np.where(topmask[None, :, :].transpose(0, 2, 1),  # [1, K, B]
                    P[:K].transpose(2, 0, 1),          # [N, K, B]
                    0.0).astype(np.float32)

    # bottom: V[64:128, k, b] = P[len_b-1-k, b, :] while k <= len_b-1-m_b
    tidx = (ln[None, :] - 1 - kk[:, None])             # [K, B] time index
    botmask = kk[:, None] <= (ln - 1 - m)[None, :]     # [K, B]
    tclip = np.clip(tidx, 0, T - 1)
    Pg = np.take_along_axis(P, tclip[:, :, None], axis=0)  # [K, B, N]
    Vbot = np.where(botmask[:, :, None], Pg, 0.0).transpose(2, 0, 1).astype(np.float32)

    Vfull = np.concatenate([Vtop, Vbot], axis=0)       # [128, K, B]

    Cp = np.zeros((2 * N, K, B), np.uint8)
    bidx = np.arange(B)
    Cp[:N, m - 1, bidx] = 1
    kb = ln - 1 - m
    has_b = kb >= 0
    Cp[N:, kb[has_b], bidx[has_b]] = 1

    Ef = np.exp(tr).astype(np.float32)                 # [to, fr]
    E2 = np.zeros((2 * N, 2 * N), np.float32)
    E2[:N, :N] = Ef.T                                  # S[fr, to] = Ef[to, fr]
    E2[N:, N:] = Ef                                    # S[to, fr] = Ef[to, fr]

    X0 = np.zeros((2 * N, BC), np.float32)
    X0[:N, :] = Ef[:, START_IDX][:, None]              # F_0
    X0[N:, :] = Ef[END_IDX, :][:, None]                # g^(len) = w

    # C_b = sum_{t < len_b} r[t, b]
    tmask = np.arange(T)[:, None] < ln[None, :]
    C = (r * tmask).sum(axis=0)                        # [B] f64

    return Vfull, Cp, E2, X0, C, tr, ln


def _host_finish(Yc_all, tr, ln, C):
    """Yc_all: [B_total? ...] list per core of [128, BC] -> logZ [B]."""
    Ef64 = np.exp(tr.astype(np.float64))               # [to, fr]
    w64 = Ef64[END_IDX, :]
    out = np.zeros(B, np.float64)
    for core, Yc in enumerate(Yc_all):
        top = Yc[:N].astype(np.float64)                # [N, BC] f_m
        bot = Yc[N:].astype(np.float64)                # [N, BC]
        g = Ef64.T @ bot                               # [N, BC] g^(m) (len >= 2)
        z = (top * g).sum(axis=0)
        z1 = (top * w64[:, None]).sum(axis=0)          # len == 1 case
        cb = slice(core * BC, (core + 1) * BC)
        lncb = ln[cb]
        zz = np.where(lncb == 1, z1, z)
        out[cb] = np.log(zz) + C[cb]
    return out.astype(np.float32)


def _build_in_maps(unary, trans, lengths):
    Vfull, Cp, E2, X0, C, tr, ln = _host_prep(unary, trans, lengths)
    in_maps = []
    for core in range(NCORES):
        cb = slice(core * BC, (core + 1) * BC)
        v_sb = np.ascontiguousarray(Vfull[:, :, cb].reshape(2 * N, K * BC))
        cp_sb = np.ascontiguousarray(Cp[:, :, cb].reshape(2 * N, K * BC))
        in_maps.append({"v": v_sb, "cp": cp_sb, "e2": E2, "x0": X0})
    return in_maps, (tr, ln, C)


def _finish(core_outs, aux):
    """core_outs: list of per-core result dicts -> full [B] output."""
    tr, ln, C = aux
    Yc_all = [core_outs[i]["out"].reshape(2 * N, BC) for i in range(NCORES)]
    return _host_finish(Yc_all, tr, ln, C)


def kernel(unary, trans, lengths):
    from concourse.bass_utils import run_bass_kernel_spmd

    in_maps, aux = _build_in_maps(unary, trans, lengths)
    nc = _build_nc()
    res = run_bass_kernel_spmd(nc, in_maps, list(range(NCORES)))
    return _finish(res.results, aux)
